# revision 16
# baseline (speedup 1.0000x reference)
"""Trainium2 Bass kernel for nn_MeanDegConv (gnn_message_passing) on 8 NeuronCores.

Self-contained: imports the Bass/Tile stack from /opt/trn_rl_repo (part of the
container environment) and hardcodes all shapes/sharding for the problem.

Design: fold the MLPs algebraically so on-device work is two gather+segment-sum
stages (edge means, then vertex means) plus small dense matmuls.
  stage1: S[e] = sum_{i: edges[i]=e} X[vertex[i]];  xe = (S@K1)/deg_e + logdeg*k2 + c1
  allgather xe across cores (edge-sharded -> replicated)
  stage2: Z[v] = sum_{i: vertex[i]=v} xe[edges[i]];  out = relu(Z/deg_v + X@MX + X0@MX0
          + logdeg_v*r4 + c0) @ W3w2 + b2
Segment sums run as one-hot matmuls over gathered rows (gather via SWDGE
dma_gather, descriptors spread over 4 SWDGE queues = 4 Q7 core pairs).
All gathered data, one-hots and big matmuls are bf16 (psum accumulation f32).
"""
import sys
for _p in ('/opt/trn_rl_repo',):
    if _p not in sys.path:
        sys.path.insert(0, _p)

import numpy as np
import ml_dtypes

import concourse.bass as bass
import concourse.mybir as mybir
import concourse.tile as tile
import concourse.bacc as bacc
from concourse.bass_utils import run_bass_kernel_spmd

N, E, NNZ, D = 50000, 10000, 1000000, 128
C = 8
EPC, VPC = E // C, N // C          # 1250 edges, 6250 vertices per core
NWIN_E = (EPC + 127) // 128        # 10
NWIN_V = (VPC + 127) // 128        # 49
EP = NWIN_E * 128                  # 1280 padded edge slots per core
VP = NWIN_V * 128                  # 6272 padded vertex slots per core
CHUNK = 2048                       # gather indices per dma_gather call (129
                                   # descs/engine fits the SWDGE ring; bigger
                                   # chunks stall dispatch on ring space)
TPC = CHUNK // 128                 # 16 tiles per chunk
SPLIT = 32768                      # int16 index limit for the X table
WIDE = 4                           # one-hot tiles built per DVE op

F32 = mybir.dt.float32
BF16 = mybir.dt.bfloat16
I16 = mybir.dt.int16
BF = ml_dtypes.bfloat16

NUM_Q = 4                          # SWDGE queues (4 Q7 core pairs)


def _pack_idx16(idx32: np.ndarray) -> np.ndarray:
    """[L] int32 -> [128, L/16] int16 in the dma_gather wrap layout."""
    L = len(idx32)
    assert L % 16 == 0
    a = idx32.astype(np.int16).reshape(L // 16, 16).T  # [16, L/16]
    return np.ascontiguousarray(np.tile(a, (8, 1)))    # [128, L/16]


def _pad_to(arr, L, fill):
    out = np.full(L, fill, arr.dtype)
    out[:len(arr)] = arr
    return out


def _build_stream(per_win_idx, per_win_lidx, tiles_per_win):
    """Concatenate per-window (idx, lidx) entries, padding each window to
    tiles_per_win[w]*128 entries (idx pad 0, lidx pad -1). Returns idx
    [Lt], lidx [Lt] with Lt = sum(tiles)*128 padded to CHUNK multiple."""
    idx_parts, lidx_parts = [], []
    for w, T in enumerate(tiles_per_win):
        L = T * 128
        idx_parts.append(_pad_to(per_win_idx[w], L, 0))
        lidx_parts.append(_pad_to(per_win_lidx[w], L, -1.0))
    idx = np.concatenate(idx_parts) if idx_parts else np.zeros(0, np.int32)
    lidx = np.concatenate(lidx_parts) if lidx_parts else np.zeros(0, np.float32)
    Lt = ((len(idx) + CHUNK - 1) // CHUNK) * CHUNK
    return _pad_to(idx, Lt, 0), _pad_to(lidx, Lt, -1.0)


def prepare(inputs):
    """Host-side preprocessing: consts, per-core streams, schedule."""
    X = np.asarray(inputs["X"], np.float32)
    X0 = np.asarray(inputs["X0"], np.float32)
    v = np.asarray(inputs["vertex"]).astype(np.int64)
    e = np.asarray(inputs["edges"]).astype(np.int64)
    W1_w = np.asarray(inputs["W1_w"], np.float32); W1_b = np.asarray(inputs["W1_b"], np.float32)
    W2_w = np.asarray(inputs["W2_w"], np.float32); W2_b = np.asarray(inputs["W2_b"], np.float32)
    W3_w1 = np.asarray(inputs["W3_w1"], np.float32); W3_b1 = np.asarray(inputs["W3_b1"], np.float32)
    W3_w2 = np.asarray(inputs["W3_w2"], np.float32); W3_b2 = np.asarray(inputs["W3_b2"], np.float32)

    deg_e = np.bincount(e, minlength=E).astype(np.float32)
    deg_v = np.bincount(v, minlength=N).astype(np.float32)

    # ---- folded weight matrices (float64 for accuracy, cast at the end)
    W2a = W2_w[:D].astype(np.float64); W2b1 = W2_w[D:2*D].astype(np.float64)
    w2b_log = W2_w[2*D].astype(np.float64)
    R1 = W3_w1[:D].astype(np.float64); R2 = W3_w1[D:2*D].astype(np.float64)
    R3 = W3_w1[2*D:3*D].astype(np.float64); r4 = W3_w1[3*D].astype(np.float64)
    W2bR = W2b1 @ R1
    K1 = (W1_w.astype(np.float64) @ W2bR).astype(np.float32)
    k2 = (w2b_log @ R1).astype(np.float32)
    c1 = (W1_b.astype(np.float64) @ W2bR).astype(np.float32)
    MX = (W2a @ R1 + R2).astype(np.float32)
    MX0 = R3.astype(np.float32)
    c0 = (W2_b.astype(np.float64) @ R1 + W3_b1).astype(np.float32)

    iota = np.tile(np.arange(128, dtype=np.float32), (128, 1))
    iota4 = np.tile(np.arange(128, dtype=np.float32), (128, WIDE, 1))
    consts = {
        "iota": np.ascontiguousarray(iota.astype(BF)),
        "iota4": np.ascontiguousarray(iota4.astype(BF)),
        "K1": K1,
        "K2": np.ascontiguousarray(np.stack([k2, c1])),            # [2,128]
        "MX": np.ascontiguousarray(MX.astype(BF)),
        "MX0": np.ascontiguousarray(MX0.astype(BF)),
        "RC2": np.ascontiguousarray(np.stack([r4.astype(np.float32), c0]).astype(BF)),  # [2,128]
        "W3w2": W3_w2,
        "b2row": W3_b2.reshape(1, D),
        "ones1": np.ones((1, 128), np.float32),
        "Xtab": np.ascontiguousarray(X.astype(BF)),                # bf16 gather table
    }

    # ---- stage-1: per (core, window, half) incidence lists
    core1 = (e // EPC).astype(np.int64)          # owning core by edge range
    win1 = ((e % EPC) // 128).astype(np.int64)   # window within core
    lidx1 = ((e % EPC) % 128).astype(np.float32) # slot within window
    half1 = (v >= SPLIT).astype(np.int64)

    # bucket sort indices by (core, window, half)
    key1 = (core1 * NWIN_E + win1) * 2 + half1
    order1 = np.argsort(key1, kind="stable")
    ks = key1[order1]
    bounds1 = np.searchsorted(ks, np.arange(C * NWIN_E * 2 + 1))

    def seg1(c, w, h):
        b = (c * NWIN_E + w) * 2 + h
        s = order1[bounds1[b]:bounds1[b + 1]]
        return s[np.argsort(v[s], kind="stable")]   # ascending table rows

    cnt1 = np.diff(bounds1).reshape(C, NWIN_E, 2)
    TA = [int(np.ceil(cnt1[:, w, 0].max() / 128)) for w in range(NWIN_E)]
    TB = [int(np.ceil(cnt1[:, w, 1].max() / 128)) for w in range(NWIN_E)]

    # ---- stage-2: per (core, window) lists, indices are padded xe row ids
    core2 = (v // VPC).astype(np.int64)
    win2 = ((v % VPC) // 128).astype(np.int64)
    lidx2 = ((v % VPC) % 128).astype(np.float32)
    # row in xe_all with per-window allgather layout: [win][core][slot]
    rowid2 = ((e % EPC) // 128) * (C * 128) + (e // EPC) * 128 + (e % EPC) % 128

    key2 = core2 * NWIN_V + win2
    order2 = np.argsort(key2, kind="stable")
    ks2 = key2[order2]
    bounds2 = np.searchsorted(ks2, np.arange(C * NWIN_V + 1))

    def seg2(c, w):
        b = c * NWIN_V + w
        s = order2[bounds2[b]:bounds2[b + 1]]
        return s[np.argsort(rowid2[s], kind="stable")]

    cnt2 = np.diff(bounds2).reshape(C, NWIN_V)
    T2 = [int(np.ceil(cnt2[:, w].max() / 128)) for w in range(NWIN_V)]

    sched = {"TA": TA, "TB": TB, "T2": T2}

    # ---- per-core input maps
    in_maps = []
    log_deg_e = np.log(deg_e); log_deg_v = np.log(deg_v)
    for c in range(C):
        # stage-1 streams
        idxA = [v[seg1(c, w, 0)].astype(np.int32) for w in range(NWIN_E)]
        lidA = [lidx1[seg1(c, w, 0)] for w in range(NWIN_E)]
        idxB = [(v[seg1(c, w, 1)] - SPLIT).astype(np.int32) for w in range(NWIN_E)]
        lidB = [lidx1[seg1(c, w, 1)] for w in range(NWIN_E)]
        sA_idx, sA_lid = _build_stream(idxA, lidA, TA)
        sB_idx, sB_lid = _build_stream(idxB, lidB, TB)
        # stage-2 stream
        idx2 = [rowid2[seg2(c, w)].astype(np.int32) for w in range(NWIN_V)]
        lid2 = [lidx2[seg2(c, w)] for w in range(NWIN_V)]
        s2_idx, s2_lid = _build_stream(idx2, lid2, T2)

        # per-core edge aux (padded slots get deg=1, log=0)
        de = np.ones(EP, np.float32); de[:EPC] = deg_e[c*EPC:(c+1)*EPC]
        le = np.zeros(EP, np.float32); le[:EPC] = log_deg_e[c*EPC:(c+1)*EPC]
        auxe = np.ascontiguousarray(np.stack([de * le, de]))        # [2, EP]
        invdeg_e_col = np.ascontiguousarray(
            (1.0 / de).reshape(NWIN_E, 128).T)                      # [128, NWIN_E]

        dv = np.ones(VP, np.float32); dv[:VPC] = deg_v[c*VPC:(c+1)*VPC]
        lv = np.zeros(VP, np.float32); lv[:VPC] = log_deg_v[c*VPC:(c+1)*VPC]
        auxv = np.ascontiguousarray(np.stack([lv, np.ones(VP, np.float32)]).astype(BF))  # [2, VP]
        invdeg_bc = np.ascontiguousarray(
            np.tile(1.0 / dv, (128, 1)))                            # [128, VP]

        Xp = np.zeros((VP, D), np.float32); Xp[:VPC] = X[c*VPC:(c+1)*VPC]
        X0p = np.zeros((VP, D), np.float32); X0p[:VPC] = X0[c*VPC:(c+1)*VPC]

        m = dict(consts)
        m.update({
            "idxA": _pack_idx16(sA_idx), "lidA": np.ascontiguousarray(
                sA_lid.reshape(-1, 128).T.astype(BF)),
            "idxB": _pack_idx16(sB_idx), "lidB": np.ascontiguousarray(
                sB_lid.reshape(-1, 128).T.astype(BF)),
            "idx2": _pack_idx16(s2_idx), "lid2": np.ascontiguousarray(
                s2_lid.reshape(-1, 128).T.astype(BF)),
            "auxe": auxe, "invdeg_e_col": invdeg_e_col,
            "auxv": auxv, "invdeg_bc": invdeg_bc,
            "XT": np.ascontiguousarray(Xp.T.astype(BF)),
            "X0T": np.ascontiguousarray(X0p.T.astype(BF)),
        })
        in_maps.append(m)
    return in_maps, sched


def build(in_map0, sched, mode="full"):
    """Build the SPMD Bass program. in_map0 supplies shapes."""
    TA, TB, T2 = sched["TA"], sched["TB"], sched["T2"]
    nc = bacc.Bacc(None, num_swdge_queues=NUM_Q)

    def param(name, dt=F32):
        arr = in_map0[name]
        return nc.declare_dram_parameter(name, list(arr.shape), dt, isOutput=False)

    Xtab_d = param("Xtab", BF16)
    iota_d = param("iota", BF16); iota4_d = param("iota4", BF16)
    K1_d = param("K1"); K2_d = param("K2")
    MX_d = param("MX", BF16); MX0_d = param("MX0", BF16); RC2_d = param("RC2", BF16)
    W3w2_d = param("W3w2"); b2row_d = param("b2row"); ones1_d = param("ones1")
    idxA_d = param("idxA", I16); lidA_d = param("lidA", BF16)
    idxB_d = param("idxB", I16); lidB_d = param("lidB", BF16)
    idx2_d = param("idx2", I16); lid2_d = param("lid2", BF16)
    auxe_d = param("auxe"); invde_d = param("invdeg_e_col")
    auxv_d = param("auxv", BF16); invbc_d = param("invdeg_bc")
    XT_d = param("XT", BF16); X0T_d = param("X0T", BF16)
    out_d = nc.declare_dram_parameter("out", [VP, D], F32, isOutput=True)

    LA = in_map0["idxA"].shape[1] * 16
    LB = in_map0["idxB"].shape[1] * 16
    L2 = in_map0["idx2"].shape[1] * 16
    nchA, nchB, nch2 = LA // CHUNK, LB // CHUNK, L2 // CHUNK

    qctr = [1]   # start at 1: queue-0 gathers block the Pool engine for the
                 # full descriptor-gen; queues 1-3 dispatch in ~70ns

    def next_q():
        q = qctr[0] % NUM_Q
        qctr[0] += 1
        return q

    with tile.TileContext(nc) as tc:
        with (
            tc.tile_pool(name="const", bufs=1) as cp,
            tc.tile_pool(name="stream", bufs=1) as sp,
            tc.tile_pool(name="gA", bufs=6) as gAp,
            tc.tile_pool(name="gB", bufs=4) as gBp,
            tc.tile_pool(name="g2", bufs=6) as g2p,
            tc.tile_pool(name="pw", bufs=6) as pwp,
            tc.tile_pool(name="work", bufs=3) as wp,
            tc.tile_pool(name="fw", bufs=4) as fwp,
            tc.tile_pool(name="acc", bufs=1) as accp,
            tc.tile_pool(name="psS", bufs=1, space="PSUM") as psS,
            tc.tile_pool(name="psXE", bufs=1, space="PSUM") as psXE,
            tc.tile_pool(name="psT", bufs=1, space="PSUM") as psT,
            tc.tile_pool(name="psR", bufs=2, space="PSUM") as psR,
            tc.tile_pool(name="psO", bufs=1, space="PSUM") as psO,
            tc.tile_pool(name="dram", bufs=1, space="DRAM") as dp,
        ):
            # ---- load constants / streams
            def load(pool, dram_ap, name, dt=F32, eng=None):
                t = pool.tile(list(dram_ap.shape), dt, name=name, tag=name)
                (eng or nc.sync).dma_start(t[:], dram_ap[:])
                return t

            iota_t = load(cp, iota_d, "iota", BF16)
            iota4_t = load(cp, iota4_d, "iota4", BF16)
            K1_t = load(cp, K1_d, "K1"); K2_t = load(cp, K2_d, "K2")
            MX_t = load(cp, MX_d, "MX", BF16); MX0_t = load(cp, MX0_d, "MX0", BF16)
            RC2_t = load(cp, RC2_d, "RC2", BF16)
            W3w2_t = load(cp, W3w2_d, "W3w2"); b2row_t = load(cp, b2row_d, "b2row")
            ones1_t = load(cp, ones1_d, "ones1")
            auxe_t = load(cp, auxe_d, "auxe"); invde_t = load(cp, invde_d, "invde")
            auxv_t = load(cp, auxv_d, "auxv", BF16)
            idxA_t = load(sp, idxA_d, "idxA", I16)
            lidA_t = load(sp, lidA_d, "lidA", BF16)
            idxB_t = load(sp, idxB_d, "idxB", I16)
            lidB_t = load(sp, lidB_d, "lidB", BF16)
            idx2_t = load(sp, idx2_d, "idx2", I16)
            lid2_t = load(sp, lid2_d, "lid2", BF16)

            xe_local = dp.tile([EP, D], BF16)
            xe_w = [dp.tile([C * 128, D], BF16, addr_space="Shared",
                            name=f"xe_w{w}") for w in range(NWIN_E)]
            xe_all = dp.tile([NWIN_E * C * 128, D], BF16)

            sA_sb = accp.tile([128, EP], F32)   # S^T accumulated (pass A, then +B)

            # ================= stage 1 =================
            PRE = 3   # chunks to prefetch ahead of consumption

            class Stream:
                def __init__(self, pool, tag, idx_t, lid_t, in_ap, nch, total):
                    self.pool, self.tag = pool, tag
                    self.idx_t, self.lid_t = idx_t, lid_t
                    self.in_ap, self.nch, self.total = in_ap, nch, total
                    self.chunks = {}
                    self.next_issue = 0
                    self.tc = 0
                    self.pws = {}

                def _issue(self):
                    ci = self.next_issue
                    g = self.pool.tile([128, TPC, D], BF16, tag=self.tag,
                                       name=f"{self.tag}{ci}")
                    nc.gpsimd.dma_gather(
                        out_ap=g[:], in_ap=self.in_ap,
                        idxs_ap=self.idx_t[:, ci * (CHUNK // 16):(ci + 1) * (CHUNK // 16)],
                        num_idxs=CHUNK, num_idxs_reg=CHUNK,
                        single_packet=False, elem_size=D, queue_num=next_q())
                    self.chunks[ci] = g
                    self.next_issue += 1

                def tile(self):
                    ci = self.tc // TPC
                    while self.next_issue <= min(ci + PRE, self.nch - 1):
                        self._issue()
                    g = self.chunks[ci][:, self.tc % TPC, :]
                    grp = self.tc // WIDE
                    if grp not in self.pws:
                        base = grp * WIDE
                        nwide = min(WIDE, self.total - base)
                        pw = pwp.tile([128, WIDE, 128], BF16, tag="pw",
                                      name=f"pw{self.tag}{grp}")
                        lcols = self.lid_t[:, base:base + nwide]
                        nc.vector.tensor_tensor(
                            out=pw[:, :nwide, :],
                            in0=iota4_t[:, :nwide, :],
                            in1=lcols.unsqueeze(2).broadcast_to([128, nwide, 128]),
                            op=mybir.AluOpType.is_equal)
                        self.pws[grp] = pw
                    p = self.pws[grp][:, (self.tc % WIDE), :]
                    self.tc += 1
                    return g, p

            sA = Stream(gAp, "gA", idxA_t, lidA_t, Xtab_d[0:SPLIT, :],
                        nchA, sum(TA))
            sB = Stream(gBp, "gB", idxB_t, lidB_t, Xtab_d[SPLIT:N, :],
                        nchB, sum(TB))

            for w in range(NWIN_E):
                T = TA[w] + TB[w]
                ps0 = psS.tile([128, 128], F32, tag="s1a", name=f"ps0w{w}")
                ps1 = (psS.tile([128, 128], F32, tag="s1b", name=f"ps1w{w}")
                       if T > 1 else None)
                pp = [ps0, ps1]
                j = 0
                for _ in range(TA[w]):
                    g, p = sA.tile()
                    nc.tensor.matmul(pp[j % 2][:], g, p,
                                     start=(j < 2), stop=(j >= T - 2))
                    j += 1
                for _ in range(TB[w]):
                    g, p = sB.tile()
                    nc.tensor.matmul(pp[j % 2][:], g, p,
                                     start=(j < 2), stop=(j >= T - 2))
                    j += 1
                sl = sA_sb[:, w * 128:(w + 1) * 128]
                nc.scalar.copy(sl, ps0[:])
                if T > 1:
                    nc.vector.tensor_tensor(out=sl, in0=sl, in1=ps1[:],
                                            op=mybir.AluOpType.add)

            # xe_hat per window: psum = S^T.T@K1 + auxe.T@K2, scale by 1/deg
            for w in range(NWIN_E):
                ps = psXE.tile([128, 128], F32, tag="xe")
                nc.tensor.matmul(ps[:], sA_sb[:, w * 128:(w + 1) * 128], K1_t[:],
                                 start=True, stop=False)
                nc.tensor.matmul(ps[:], auxe_t[:, w * 128:(w + 1) * 128],
                                 K2_t[:], start=False, stop=True)
                xe_sb = wp.tile([128, D], BF16, tag="xe_sb")
                nc.scalar.activation(
                    out=xe_sb[:], in_=ps[:],
                    func=mybir.ActivationFunctionType.Copy,
                    scale=invde_t[:, w:w + 1])
                nc.sync.dma_start(xe_local[w * 128:(w + 1) * 128, :], xe_sb[:])
                if mode in ("s1ag", "full"):
                    nc.gpsimd.collective_compute(
                        "AllGather", mybir.AluOpType.bypass,
                        replica_groups=[list(range(C))],
                        ins=[xe_local[w * 128:(w + 1) * 128, :].opt()],
                        outs=[xe_w[w].opt()])
                    nc.sync.dma_start(
                        xe_all[w * C * 128:(w + 1) * C * 128, :], xe_w[w][:])

            if mode == "s1":
                for w in range(NWIN_E):
                    xe_rd = wp.tile([128, D], BF16, tag="xe_rd", name="xe_rd")
                    nc.sync.dma_start(xe_rd[:], xe_local[w * 128:(w + 1) * 128, :])
                    o32 = wp.tile([128, D], F32, tag="o32", name="o32")
                    nc.vector.tensor_scalar(
                        out=o32[:], in0=xe_rd[:], scalar1=0.0, scalar2=None,
                        op0=mybir.AluOpType.add)
                    nc.sync.dma_start(out_d[w * 128:(w + 1) * 128, :], o32[:])
            if mode == "full":
                # ================= stage 2 =================
                state2 = {}
                def consume2(w, t, T, g_sl, p_sl):
                    if t == 0:
                        state2["ps"] = [
                            psT.tile([128, 128], F32, tag="t3a", name="psT0"),
                            psT.tile([128, 128], F32, tag="t3b", name="psT1")
                            if T > 1 else None]
                    ps = state2["ps"][t % 2]
                    nc.tensor.matmul(ps[:], g_sl, p_sl,
                                     start=(t < 2), stop=(t >= T - 2))
                    if t == T - 1:
                        finish_window(w, state2["ps"][0],
                                      state2["ps"][1] if T > 1 else None)

                def finish_window(w, ps0, ps1):
                    sl = slice(w * 128, (w + 1) * 128)
                    xt = fwp.tile([128, 128], BF16, tag="xt", name="xt")
                    x0t = fwp.tile([128, 128], BF16, tag="x0t", name="x0t")
                    invbc = fwp.tile([128, 128], F32, tag="invbc", name="invbc")
                    nc.sync.dma_start(xt[:], XT_d[:, sl])
                    nc.sync.dma_start(x0t[:], X0T_d[:, sl])
                    nc.sync.dma_start(invbc[:], invbc_d[:, sl])
                    psr = psR.tile([128, 128], F32, tag="r", name="psr")
                    nc.tensor.matmul(psr[:], MX_t[:], xt[:], start=True, stop=False)
                    nc.tensor.matmul(psr[:], MX0_t[:], x0t[:], start=False, stop=False)
                    nc.tensor.matmul(psr[:], RC2_t[:], auxv_t[:, sl], start=False, stop=True)
                    pre = wp.tile([128, 128], F32, tag="pre", name="pre")
                    nc.vector.tensor_tensor(out=pre[:], in0=ps0[:], in1=invbc[:],
                                            op=mybir.AluOpType.mult)
                    if ps1 is not None:
                        tmp = wp.tile([128, 128], F32, tag="tmp", name="tmp")
                        nc.vector.tensor_tensor(out=tmp[:], in0=ps1[:], in1=invbc[:],
                                                op=mybir.AluOpType.mult)
                        nc.vector.tensor_tensor(out=pre[:], in0=pre[:], in1=tmp[:],
                                                op=mybir.AluOpType.add)
                    nc.vector.tensor_tensor(out=pre[:], in0=pre[:], in1=psr[:],
                                            op=mybir.AluOpType.add)
                    relu = wp.tile([128, 128], F32, tag="relu", name="relu")
                    nc.scalar.activation(out=relu[:], in_=pre[:],
                                         func=mybir.ActivationFunctionType.Relu)
                    pso = psO.tile([128, 128], F32, tag="o", name="pso")
                    nc.tensor.matmul(pso[:], relu[:], W3w2_t[:], start=True, stop=False)
                    nc.tensor.matmul(pso[:], ones1_t[:], b2row_t[:], start=False, stop=True)
                    o_sb = wp.tile([128, D], F32, tag="o_sb", name="o_sb")
                    nc.scalar.copy(o_sb[:], pso[:])
                    nc.sync.dma_start(out_d[w * 128:(w + 1) * 128, :], o_sb[:])

                s2 = Stream(g2p, "g2", idx2_t, lid2_t, xe_all[:],
                            nch2, sum(T2))
                for w in range(NWIN_V):
                    T = T2[w]
                    for t in range(T):
                        g, p = s2.tile()
                        consume2(w, t, T, g, p)

    nc.finalize()
    return nc


def run(trace=False, mode="full", **inputs):
    in_maps, sched = prepare(inputs)
    nc = build(in_maps[0], sched, mode=mode)
    res = run_bass_kernel_spmd(nc, in_maps, list(range(C)), trace=trace)
    out = np.concatenate([res.results[c]["out"][:VPC] for c in range(C)], axis=0)
    return out, res


def kernel(**inputs):
    """Harness entry point: full inputs in, full [N, D] float32 output."""
    out, _res = run(trace=False, mode="full", **inputs)
    return out.astype(np.float32)
